# revision 18
# baseline (speedup 1.0000x reference)
"""Trainium2 Bass kernel for nn_DecSwitchedDeconv (switched per-sample double deconv).

Strategy (data-parallel over 8 cores, 32 samples/core, processed in pairs):
  - fp8(e4m3) DoubleRow matmuls: each matmul contracts K=256 (two k-tiles
    summed in one pass) at the same per-column cost as a bf16 K=128 matmul,
    halving tensor-engine work vs bf16. k-tile column strides must be even.
  - conv1 (row-aligned chunks of 11/11/10 image rows): 5 DR matmuls per chunk
    cover 9 taps via tap pairs (delta=2 within a kernel row, delta=34 across);
    conv2: 3 DR matmuls per chunk using the row-shifted hpad duplicate
    (dy0/dy1 in partitions 0:64/64:128, dy2 via +34 column offset with
    zero-masked duplicate taps).
  - Row-aligned chunks let PSUM be read with strided APs that skip the pad
    columns: no pad-column fixup pass, and the conv2 epilogue writes the
    unpadded output tile directly (no opad). Guard columns (+2 front, +4
    back) absorb edge-tap reads of junk output columns.
  - Per-sample branch weights gathered on the HOST into per-pair
    block-diagonal fp8 tables (one contiguous DMA per pair per conv); z is
    folded into W2 and b2*z; biases for all pairs preloaded in one DMA.
  - relu(x) precomputed on the host as pre-padded fp8 rows and DMA'd
    straight into xpad's interior; x also loaded as bf16 for the residual;
    output stored bf16. ScalarE: 3x (bias+relu psum->fp8). VectorE: 3x
    epilogue (psum+b2z) + residual add. hpad row-shift duplicate via
    SBUF->SBUF DMA. Loads + copies on the sync queue (HW DGE), bulk stores
    on the gpsimd queue; PE p-state warmed with dummy matmuls at startup;
    one-iteration software pipeline (conv1(p) overlaps conv2(p-1)).
"""

import numpy as np

import concourse.bacc as bacc
import concourse.bass as bass
import concourse.mybir as mybir
import concourse.tile as tile
from concourse.bass_utils import run_bass_kernel_spmd

B, C, CSM, NB, HW = 256, 64, 32, 8, 32
M = 8                  # cores
BS = B // M            # 32 samples per core
NPAIR = BS // 2        # 16
WP = HW + 2            # 34 padded width
L = WP * WP            # 1156
GD = 2                 # guard columns before the padded image
L2 = L + 6             # guarded tensor width (zeros outside the image)
NBUF = 4               # ping-pong depth for persistent per-pair buffers
CHUNKS = [(1, 11), (12, 11), (23, 10)]   # (first image row, rows) per chunk

f32 = mybir.dt.float32
bf16 = mybir.dt.bfloat16
fp8 = mybir.dt.float8e4

# conv1 DR slots: tap pair (tap0 or None, tap1); rhs tile base is derived
# from tap0's (dy,dx); k-tile delta must be EVEN (hw constraint).
C1SLOTS = [
    (((0, 0), (0, 2)), (0, 0), 2),
    (((1, 0), (1, 2)), (1, 0), 2),
    (((2, 0), (2, 2)), (2, 0), 2),
    (((0, 1), (1, 1)), (0, 1), 34),
    ((None, (2, 1)), (1, 1), 34),   # tile0 zero-weighted
]
# conv2 DR slots: (tile0 lo-position (dy,dx), delta); tiles cover lo tap at
# rows 0:64 and lo+(1,0) at rows 64:128 (the shifted hpad copy)
C2SLOTS = [((0, 0), 2), ((0, 1), 34), ((1, 0), 2)]
# per-tile (lo_tap or None if duplicate-masked, hi_tap)
C2TILES = [((0, 0), (1, 0)), ((0, 2), (1, 2)),
           ((0, 1), (1, 1)), (None, (2, 1)),
           (None, (2, 0)), (None, (2, 2))]


def _build_bass():
    nc = bacc.Bacc(target_bir_lowering=False, debug=False)
    xs = nc.dram_tensor("xs", [BS * C, HW * HW], bf16, kind="ExternalInput")
    xr = nc.dram_tensor("xr", [BS * C, WP * HW], fp8, kind="ExternalInput")
    w1p = nc.dram_tensor("w1p", [NPAIR * 128, 5 * 2 * 64], fp8, kind="ExternalInput")
    w2p = nc.dram_tensor("w2p", [NPAIR * 128, 6 * 128], fp8, kind="ExternalInput")
    b1p = nc.dram_tensor("b1p", [64, NPAIR], f32, kind="ExternalInput")
    b2zp = nc.dram_tensor("b2zp", [128, NPAIR], f32, kind="ExternalInput")
    outd = nc.dram_tensor("out", [BS * C, HW * HW], bf16, kind="ExternalOutput")

    add = mybir.AluOpType.add
    Relu = mybir.ActivationFunctionType.Relu
    DR = mybir.MatmulPerfMode.DoubleRow

    with tile.TileContext(nc) as tc:
        b1sb = nc.alloc_sbuf_tensor("b1sb", [64, NPAIR], f32).ap()
        b2zsb = nc.alloc_sbuf_tensor("b2zsb", [128, NPAIR], f32).ap()

        # zero scratch for PE p-state warm-up matmuls
        zdum = nc.alloc_sbuf_tensor("zdum", [128, 384], fp8).ap()
        nc.vector.memset(zdum, 0.0)

        xpads, hpads = [], []
        for i in range(NBUF):
            xpads.append(nc.alloc_sbuf_tensor(f"xpad{i}", [128, L2], fp8).ap())
            hpads.append(nc.alloc_sbuf_tensor(f"hpad{i}", [128, L2], fp8).ap())
            nc.vector.memset(xpads[i], 0.0)
            nc.vector.memset(hpads[i], 0.0)

        with (
            tc.tile_pool(name="io", bufs=4) as iop,
            tc.tile_pool(name="psw", bufs=1, space="PSUM") as pswp,
            tc.tile_pool(name="ps1", bufs=3, space="PSUM") as ps1p,
            tc.tile_pool(name="ps2", bufs=3, space="PSUM") as ps2p,
        ):
            # warm the tensor engine while the first DMAs are in flight
            psw = pswp.tile([64, 376], f32, tag="psw")
            zl = zdum[:, 0:128].rearrange("p (two m) -> p two m", two=2)
            for _ in range(14):
                zr = bass.AP(zdum.tensor, 0, [[384, 128], [2, 2], [1, 376]])
                nc.tensor.matmul(psw[:, :], lhsT=zl, rhs=zr,
                                 start=True, stop=True, perf_mode=DR)

            state = {}

            def emit_dma(p):
                bi = p % NBUF
                xpad, hpad = xpads[bi], hpads[bi]
                xst = iop.tile([128, HW * HW], bf16, tag="xst")
                wt1 = iop.tile([128, 5 * 2 * 64], fp8, tag="wt1")
                wt2 = iop.tile([128, 6 * 128], fp8, tag="wt2")
                r0 = p * 128
                # relu(x) arrives pre-padded fp8 from the host, straight into
                # xpad's interior rows (borders stay zero across buffer reuse)
                nc.sync.dma_start(xpad[:, GD + WP:GD + WP + WP * HW],
                                  xr.ap()[r0:r0 + 128, :])
                nc.sync.dma_start(wt1[:, :], w1p.ap()[r0:r0 + 128, :])
                nc.sync.dma_start(xst[:, :], xs.ap()[r0:r0 + 128, :])
                nc.sync.dma_start(wt2[:, :], w2p.ap()[r0:r0 + 128, :])
                if p == 0:
                    nc.sync.dma_start(b1sb, b1p.ap())
                    nc.sync.dma_start(b2zsb, b2zp.ap())
                xst3 = xst[:, :].rearrange("p (h w) -> p h w", w=HW)
                state[p] = (xpad, hpad, xst, wt1, wt2, xst3)

            def emit_conv1(p):
                xpad, hpad, xst, wt1, wt2, xst3 = state[p]
                hpad3 = hpad[:, GD:GD + L].rearrange("p (h w) -> p h w", w=WP)
                for (y0, rows) in CHUNKS:
                    ncol = WP * rows
                    ps1 = ps1p.tile([64, WP * 11], f32, tag="ps1")
                    for j, (_, t0, delta) in enumerate(C1SLOTS):
                        base = GD + WP * (y0 + t0[0] - 1) + t0[1] - 1
                        rhs = bass.AP(xpad.tensor, base,
                                      [[L2, 128], [delta, 2], [1, ncol]])
                        lhsT = wt1[:, 128 * j:128 * (j + 1)].rearrange(
                            "p (two m) -> p two m", two=2)
                        nc.tensor.matmul(ps1[:, 0:ncol], lhsT=lhsT, rhs=rhs,
                                         start=(j == 0), stop=(j == 4),
                                         perf_mode=DR)
                    ps1v = ps1[:, 0:ncol].rearrange(
                        "p (r w) -> p r w", w=WP)[:, :, 1:HW + 1]
                    nc.scalar.activation(
                        hpad3[0:64, y0:y0 + rows, 1:HW + 1], ps1v,
                        Relu, bias=b1sb[:, p:p + 1])
                # rows 64:128 = h shifted left one image row (dy+1 taps);
                # SBUF->SBUF copy on the sync DMA queue (HW DGE)
                nc.sync.dma_start(hpad[64:128, 0:L2 - WP], hpad[0:64, WP:L2])

            def emit_conv2(p, split_tail=False):
                xpad, hpad, xst, wt1, wt2, xst3 = state.pop(p)
                otmp = iop.tile([128, HW * HW], bf16, tag="otmp")
                ot = iop.tile([128, HW * HW], bf16, tag="ot")
                ot3 = ot[:, :].rearrange("p (h w) -> p h w", w=HW)
                otmp3 = otmp[:, :].rearrange("p (h w) -> p h w", w=HW)
                for (y0, rows) in CHUNKS:
                    ncol = WP * rows
                    ps2 = ps2p.tile([128, WP * 11], f32, tag="ps2")
                    for j, (t0, delta) in enumerate(C2SLOTS):
                        base = GD + WP * (y0 + t0[0] - 1) + t0[1] - 1
                        rhs = bass.AP(hpad.tensor, base,
                                      [[L2, 128], [delta, 2], [1, ncol]])
                        lhsT = wt2[:, 256 * j:256 * (j + 1)].rearrange(
                            "p (two m) -> p two m", two=2)
                        nc.tensor.matmul(ps2[:, 0:ncol], lhsT=lhsT, rhs=rhs,
                                         start=(j == 0), stop=(j == 2),
                                         perf_mode=DR)
                    ps2v = ps2[:, 0:ncol].rearrange(
                        "p (r w) -> p r w", w=WP)[:, :, 1:HW + 1]
                    nc.vector.tensor_scalar(
                        otmp3[:, y0 - 1:y0 - 1 + rows, :], ps2v,
                        b2zsb[:, p:p + 1], None, op0=add)
                    if split_tail:
                        rs = slice(y0 - 1, y0 - 1 + rows)
                        nc.vector.tensor_tensor(
                            ot3[:, rs, :], otmp3[:, rs, :], xst3[:, rs, :],
                            op=add)
                        nc.sync.dma_start(
                            outd.ap()[p * 128:p * 128 + 128,
                                      HW * (y0 - 1):HW * (y0 - 1 + rows)],
                            ot[:, HW * (y0 - 1):HW * (y0 - 1 + rows)])
                if not split_tail:
                    nc.vector.tensor_tensor(ot3, otmp3, xst3, op=add)
                    nc.gpsimd.dma_start(outd.ap()[p * 128:p * 128 + 128, :],
                                        ot[:, :])

            # software pipeline with one-iteration prefetch: DMAs for pair p+1
            # are issued during pair p's conv work, and the tensor queue runs
            # conv1(p) then conv2(p-1), hiding the relu1+copy latency between
            # a pair's two convs
            emit_dma(0)
            for p in range(NPAIR):
                if p + 1 < NPAIR:
                    emit_dma(p + 1)
                emit_conv1(p)
                if p > 0:
                    emit_conv2(p - 1)
            emit_conv2(NPAIR - 1, split_tail=True)

    nc.compile()
    return nc


_NC = None


def _get_nc():
    global _NC
    if _NC is None:
        _NC = _build_bass()
    return _NC


def _host_prep(x, y_index, z, W1, b1, W2, b2):
    import ml_dtypes
    f8 = ml_dtypes.float8_e4m3
    # flipped kernels, tap-indexed [b, cin, dy, dx, cout]
    w1t = np.ascontiguousarray(W1[:, :, :, ::-1, ::-1].transpose(0, 1, 3, 4, 2))
    w2t = np.ascontiguousarray(W2[:, :, :, ::-1, ::-1].transpose(0, 1, 3, 4, 2))

    # conv1 per-branch slot table [NB, cin64, slot5, tile2, cout32]
    base1 = np.zeros((NB, C, 5, 2, CSM), np.float32)
    for j, (taps, _, _) in enumerate(C1SLOTS):
        for i, t in enumerate(taps):
            if t is not None:
                base1[:, :, j, i, :] = w1t[:, :, t[0], t[1], :]

    idx = y_index.reshape(B).astype(np.int64)
    out_maps = []
    for core in range(M):
        sl = slice(core * BS, (core + 1) * BS)
        idc = idx[sl]
        zc = z[sl]
        idxA, idxB = idc[0::2], idc[1::2]

        w1pairs = np.zeros((NPAIR, 128, 5, 2, 2 * CSM), np.float32)
        w1pairs[:, 0:C, :, :, 0:CSM] = base1[idxA]
        w1pairs[:, C:128, :, :, CSM:2 * CSM] = base1[idxB]
        w1pc = w1pairs.reshape(NPAIR * 128, 640).astype(f8)

        # conv2 per-sample z-folded tables [BS, csm32, tile6, cout64]
        w2g = w2t[idc] * zc[:, None, None, None, :]   # [BS, 32, 3, 3, 64]
        lo = np.zeros((BS, CSM, 6, C), np.float32)
        hi = np.zeros((BS, CSM, 6, C), np.float32)
        for k, (tlo, thi) in enumerate(C2TILES):
            if tlo is not None:
                lo[:, :, k, :] = w2g[:, :, tlo[0], tlo[1], :]
            hi[:, :, k, :] = w2g[:, :, thi[0], thi[1], :]
        w2pairs = np.zeros((NPAIR, 128, 6, 2 * C), np.float32)
        w2pairs[:, 0:32, :, 0:C] = lo[0::2]
        w2pairs[:, 32:64, :, C:2 * C] = lo[1::2]
        w2pairs[:, 64:96, :, 0:C] = hi[0::2]
        w2pairs[:, 96:128, :, C:2 * C] = hi[1::2]
        w2pc = w2pairs.reshape(NPAIR * 128, 768).astype(f8)

        b1pc = np.ascontiguousarray(
            np.concatenate([b1[idxA].T, b1[idxB].T], axis=0), dtype=np.float32)
        b2zc = b2[idc] * zc                            # [BS, 64]
        b2zpc = np.ascontiguousarray(
            np.concatenate([b2zc[0::2].T, b2zc[1::2].T], axis=0),
            dtype=np.float32)

        xc4 = np.ascontiguousarray(x[sl]).reshape(BS * C, HW, HW)
        xc = xc4.reshape(BS * C, HW * HW).astype(ml_dtypes.bfloat16)
        # pre-padded relu(x): rows of [0 | relu row | 0] -> 34*32 fp8 cols
        xrp = np.zeros((BS * C, HW, WP), np.float32)
        xrp[:, :, 1:HW + 1] = np.maximum(xc4, 0.0)
        xrc = xrp.reshape(BS * C, WP * HW).astype(ml_dtypes.float8_e4m3)
        out_maps.append(dict(xs=xc, xr=xrc, w1p=w1pc, w2p=w2pc,
                             b1p=b1pc, b2zp=b2zpc))
    return out_maps


def kernel(x, y_index, y_hard, z, W1, b1, W2, b2, _trace=False):
    x = np.asarray(x, dtype=np.float32)
    z = np.asarray(z, dtype=np.float32)
    y_index = np.asarray(y_index)
    W1 = np.asarray(W1, dtype=np.float32)
    b1 = np.asarray(b1, dtype=np.float32)
    W2 = np.asarray(W2, dtype=np.float32)
    b2 = np.asarray(b2, dtype=np.float32)

    nc = _get_nc()
    in_maps = _host_prep(x, y_index, z, W1, b1, W2, b2)
    res = run_bass_kernel_spmd(nc, in_maps, core_ids=list(range(M)), trace=_trace)
    out = np.concatenate(
        [r["out"].astype(np.float32).reshape(BS, C, HW, HW) for r in res.results],
        axis=0)
    if _trace:
        kernel._last_results = res
    return out


# revision 19
# speedup vs baseline: 1.0049x; 1.0049x over previous
"""Trainium2 Bass kernel for nn_DecSwitchedDeconv (switched per-sample double deconv).

Strategy (data-parallel over 8 cores, 32 samples/core, processed in pairs):
  - fp8(e4m3) DoubleRow matmuls: each matmul contracts K=256 (two k-tiles
    summed in one pass) at the same per-column cost as a bf16 K=128 matmul,
    halving tensor-engine work vs bf16. k-tile column strides must be even.
  - conv1 (row-aligned chunks of 11/11/10 image rows): 5 DR matmuls per chunk
    cover 9 taps via tap pairs (delta=2 within a kernel row, delta=34 across);
    conv2: 3 DR matmuls per chunk using the row-shifted hpad duplicate
    (dy0/dy1 in partitions 0:64/64:128, dy2 via +34 column offset with
    zero-masked duplicate taps).
  - Row-aligned chunks let PSUM be read with strided APs that skip the pad
    columns: no pad-column fixup pass, and the conv2 epilogue writes the
    unpadded output tile directly (no opad). Guard columns (+2 front, +4
    back) absorb edge-tap reads of junk output columns.
  - Per-sample branch weights gathered on the HOST into per-pair
    block-diagonal fp8 tables (one contiguous DMA per pair per conv); z is
    folded into W2 and b2*z; biases for all pairs preloaded in one DMA.
  - relu(x) precomputed on the host as pre-padded fp8 rows and DMA'd
    straight into xpad's interior; x also loaded as bf16 for the residual;
    output stored bf16. ScalarE: 3x (bias+relu psum->fp8). VectorE: 3x
    epilogue (psum+b2z) + residual add. hpad row-shift duplicate via
    SBUF->SBUF DMA. Loads + copies on the sync queue (HW DGE), bulk stores
    on the gpsimd queue; PE p-state warmed with dummy matmuls at startup;
    one-iteration software pipeline (conv1(p) overlaps conv2(p-1)).
"""

import numpy as np

import concourse.bacc as bacc
import concourse.bass as bass
import concourse.mybir as mybir
import concourse.tile as tile
from concourse.bass_utils import run_bass_kernel_spmd

B, C, CSM, NB, HW = 256, 64, 32, 8, 32
M = 8                  # cores
BS = B // M            # 32 samples per core
NPAIR = BS // 2        # 16
WP = HW + 2            # 34 padded width
L = WP * WP            # 1156
GD = 2                 # guard columns before the padded image
L2 = L + 6             # guarded tensor width (zeros outside the image)
NBUF = 4               # ping-pong depth for persistent per-pair buffers
CHUNKS = [(1, 11), (12, 11), (23, 10)]   # (first image row, rows) per chunk

f32 = mybir.dt.float32
bf16 = mybir.dt.bfloat16
fp8 = mybir.dt.float8e4

# conv1 DR slots: tap pair (tap0 or None, tap1); rhs tile base is derived
# from tap0's (dy,dx); k-tile delta must be EVEN (hw constraint).
C1SLOTS = [
    (((0, 0), (0, 2)), (0, 0), 2),
    (((1, 0), (1, 2)), (1, 0), 2),
    (((2, 0), (2, 2)), (2, 0), 2),
    (((0, 1), (1, 1)), (0, 1), 34),
    ((None, (2, 1)), (1, 1), 34),   # tile0 zero-weighted
]
# conv2 DR slots: (tile0 lo-position (dy,dx), delta); tiles cover lo tap at
# rows 0:64 and lo+(1,0) at rows 64:128 (the shifted hpad copy)
C2SLOTS = [((0, 0), 2), ((0, 1), 34), ((1, 0), 2)]
# per-tile (lo_tap or None if duplicate-masked, hi_tap)
C2TILES = [((0, 0), (1, 0)), ((0, 2), (1, 2)),
           ((0, 1), (1, 1)), (None, (2, 1)),
           (None, (2, 0)), (None, (2, 2))]


def _build_bass():
    nc = bacc.Bacc(target_bir_lowering=False, debug=False)
    xs = nc.dram_tensor("xs", [BS * C, HW * HW], bf16, kind="ExternalInput")
    xr = nc.dram_tensor("xr", [BS * C, WP * HW], fp8, kind="ExternalInput")
    w1p = nc.dram_tensor("w1p", [NPAIR * 128, 5 * 2 * 64], fp8, kind="ExternalInput")
    w2p = nc.dram_tensor("w2p", [NPAIR * 128, 6 * 128], fp8, kind="ExternalInput")
    b1p = nc.dram_tensor("b1p", [64, NPAIR], f32, kind="ExternalInput")
    b2zp = nc.dram_tensor("b2zp", [128, NPAIR], f32, kind="ExternalInput")
    outd = nc.dram_tensor("out", [BS * C, HW * HW], bf16, kind="ExternalOutput")

    add = mybir.AluOpType.add
    Relu = mybir.ActivationFunctionType.Relu
    DR = mybir.MatmulPerfMode.DoubleRow

    with tile.TileContext(nc) as tc:
        b1sb = nc.alloc_sbuf_tensor("b1sb", [64, NPAIR], f32).ap()
        b2zsb = nc.alloc_sbuf_tensor("b2zsb", [128, NPAIR], f32).ap()

        # zero scratch for PE p-state warm-up matmuls
        zdum = nc.alloc_sbuf_tensor("zdum", [128, 384], fp8).ap()
        nc.vector.memset(zdum, 0.0)

        xpads, hpads = [], []
        for i in range(NBUF):
            xpads.append(nc.alloc_sbuf_tensor(f"xpad{i}", [128, L2], fp8).ap())
            hpads.append(nc.alloc_sbuf_tensor(f"hpad{i}", [128, L2], fp8).ap())
            nc.vector.memset(xpads[i], 0.0)
            nc.vector.memset(hpads[i], 0.0)

        with (
            tc.tile_pool(name="io", bufs=4) as iop,
            tc.tile_pool(name="psw", bufs=1, space="PSUM") as pswp,
            tc.tile_pool(name="ps1", bufs=3, space="PSUM") as ps1p,
            tc.tile_pool(name="ps2", bufs=3, space="PSUM") as ps2p,
        ):
            # warm the tensor engine while the first DMAs are in flight
            psw = pswp.tile([64, 376], f32, tag="psw")
            zl = zdum[:, 0:128].rearrange("p (two m) -> p two m", two=2)
            for _ in range(14):
                zr = bass.AP(zdum.tensor, 0, [[384, 128], [2, 2], [1, 376]])
                nc.tensor.matmul(psw[:, :], lhsT=zl, rhs=zr,
                                 start=True, stop=True, perf_mode=DR)

            state = {}

            def emit_dma(p):
                bi = p % NBUF
                xpad, hpad = xpads[bi], hpads[bi]
                xst = iop.tile([128, HW * HW], bf16, tag="xst")
                wt1 = iop.tile([128, 5 * 2 * 64], fp8, tag="wt1")
                wt2 = iop.tile([128, 6 * 128], fp8, tag="wt2")
                r0 = p * 128
                # relu(x) arrives pre-padded fp8 from the host, straight into
                # xpad's interior rows (borders stay zero across buffer reuse)
                nc.sync.dma_start(xpad[:, GD + WP:GD + WP + WP * HW],
                                  xr.ap()[r0:r0 + 128, :])
                nc.sync.dma_start(wt1[:, :], w1p.ap()[r0:r0 + 128, :])
                nc.sync.dma_start(xst[:, :], xs.ap()[r0:r0 + 128, :])
                nc.sync.dma_start(wt2[:, :], w2p.ap()[r0:r0 + 128, :])
                if p == 0:
                    nc.sync.dma_start(b1sb, b1p.ap())
                    nc.sync.dma_start(b2zsb, b2zp.ap())
                xst3 = xst[:, :].rearrange("p (h w) -> p h w", w=HW)
                state[p] = (xpad, hpad, xst, wt1, wt2, xst3)

            def emit_conv1(p):
                xpad, hpad, xst, wt1, wt2, xst3 = state[p]
                copyq = nc.sync if p >= NPAIR - 3 else nc.gpsimd
                hpad3 = hpad[:, GD:GD + L].rearrange("p (h w) -> p h w", w=WP)
                for (y0, rows) in CHUNKS:
                    ncol = WP * rows
                    ps1 = ps1p.tile([64, WP * 11], f32, tag="ps1")
                    for j, (_, t0, delta) in enumerate(C1SLOTS):
                        base = GD + WP * (y0 + t0[0] - 1) + t0[1] - 1
                        rhs = bass.AP(xpad.tensor, base,
                                      [[L2, 128], [delta, 2], [1, ncol]])
                        lhsT = wt1[:, 128 * j:128 * (j + 1)].rearrange(
                            "p (two m) -> p two m", two=2)
                        nc.tensor.matmul(ps1[:, 0:ncol], lhsT=lhsT, rhs=rhs,
                                         start=(j == 0), stop=(j == 4),
                                         perf_mode=DR)
                    ps1v = ps1[:, 0:ncol].rearrange(
                        "p (r w) -> p r w", w=WP)[:, :, 1:HW + 1]
                    nc.scalar.activation(
                        hpad3[0:64, y0:y0 + rows, 1:HW + 1], ps1v,
                        Relu, bias=b1sb[:, p:p + 1])
                # rows 64:128 = h shifted left one image row (dy+1 taps);
                # SBUF->SBUF copy on the gpsimd DMA queue mid-run (relieves
                # the sync ring), on sync for the last pairs (fast drain)
                copyq.dma_start(hpad[64:128, 0:L2 - WP], hpad[0:64, WP:L2])

            def emit_conv2(p, split_tail=False):
                xpad, hpad, xst, wt1, wt2, xst3 = state.pop(p)
                otmp = iop.tile([128, HW * HW], bf16, tag="otmp")
                ot = iop.tile([128, HW * HW], bf16, tag="ot")
                ot3 = ot[:, :].rearrange("p (h w) -> p h w", w=HW)
                otmp3 = otmp[:, :].rearrange("p (h w) -> p h w", w=HW)
                for (y0, rows) in CHUNKS:
                    ncol = WP * rows
                    ps2 = ps2p.tile([128, WP * 11], f32, tag="ps2")
                    for j, (t0, delta) in enumerate(C2SLOTS):
                        base = GD + WP * (y0 + t0[0] - 1) + t0[1] - 1
                        rhs = bass.AP(hpad.tensor, base,
                                      [[L2, 128], [delta, 2], [1, ncol]])
                        lhsT = wt2[:, 256 * j:256 * (j + 1)].rearrange(
                            "p (two m) -> p two m", two=2)
                        nc.tensor.matmul(ps2[:, 0:ncol], lhsT=lhsT, rhs=rhs,
                                         start=(j == 0), stop=(j == 2),
                                         perf_mode=DR)
                    ps2v = ps2[:, 0:ncol].rearrange(
                        "p (r w) -> p r w", w=WP)[:, :, 1:HW + 1]
                    nc.vector.tensor_scalar(
                        otmp3[:, y0 - 1:y0 - 1 + rows, :], ps2v,
                        b2zsb[:, p:p + 1], None, op0=add)
                    if split_tail:
                        rs = slice(y0 - 1, y0 - 1 + rows)
                        nc.vector.tensor_tensor(
                            ot3[:, rs, :], otmp3[:, rs, :], xst3[:, rs, :],
                            op=add)
                        nc.sync.dma_start(
                            outd.ap()[p * 128:p * 128 + 128,
                                      HW * (y0 - 1):HW * (y0 - 1 + rows)],
                            ot[:, HW * (y0 - 1):HW * (y0 - 1 + rows)])
                if not split_tail:
                    nc.vector.tensor_tensor(ot3, otmp3, xst3, op=add)
                    nc.gpsimd.dma_start(outd.ap()[p * 128:p * 128 + 128, :],
                                        ot[:, :])

            # software pipeline with one-iteration prefetch: DMAs for pair p+1
            # are issued during pair p's conv work, and the tensor queue runs
            # conv1(p) then conv2(p-1), hiding the relu1+copy latency between
            # a pair's two convs
            emit_dma(0)
            for p in range(NPAIR):
                if p + 1 < NPAIR:
                    emit_dma(p + 1)
                emit_conv1(p)
                if p > 0:
                    emit_conv2(p - 1)
            emit_conv2(NPAIR - 1, split_tail=True)

    nc.compile()
    return nc


_NC = None


def _get_nc():
    global _NC
    if _NC is None:
        _NC = _build_bass()
    return _NC


def _host_prep(x, y_index, z, W1, b1, W2, b2):
    import ml_dtypes
    f8 = ml_dtypes.float8_e4m3
    # flipped kernels, tap-indexed [b, cin, dy, dx, cout]
    w1t = np.ascontiguousarray(W1[:, :, :, ::-1, ::-1].transpose(0, 1, 3, 4, 2))
    w2t = np.ascontiguousarray(W2[:, :, :, ::-1, ::-1].transpose(0, 1, 3, 4, 2))

    # conv1 per-branch slot table [NB, cin64, slot5, tile2, cout32]
    base1 = np.zeros((NB, C, 5, 2, CSM), np.float32)
    for j, (taps, _, _) in enumerate(C1SLOTS):
        for i, t in enumerate(taps):
            if t is not None:
                base1[:, :, j, i, :] = w1t[:, :, t[0], t[1], :]

    idx = y_index.reshape(B).astype(np.int64)
    out_maps = []
    for core in range(M):
        sl = slice(core * BS, (core + 1) * BS)
        idc = idx[sl]
        zc = z[sl]
        idxA, idxB = idc[0::2], idc[1::2]

        w1pairs = np.zeros((NPAIR, 128, 5, 2, 2 * CSM), np.float32)
        w1pairs[:, 0:C, :, :, 0:CSM] = base1[idxA]
        w1pairs[:, C:128, :, :, CSM:2 * CSM] = base1[idxB]
        w1pc = w1pairs.reshape(NPAIR * 128, 640).astype(f8)

        # conv2 per-sample z-folded tables [BS, csm32, tile6, cout64]
        w2g = w2t[idc] * zc[:, None, None, None, :]   # [BS, 32, 3, 3, 64]
        lo = np.zeros((BS, CSM, 6, C), np.float32)
        hi = np.zeros((BS, CSM, 6, C), np.float32)
        for k, (tlo, thi) in enumerate(C2TILES):
            if tlo is not None:
                lo[:, :, k, :] = w2g[:, :, tlo[0], tlo[1], :]
            hi[:, :, k, :] = w2g[:, :, thi[0], thi[1], :]
        w2pairs = np.zeros((NPAIR, 128, 6, 2 * C), np.float32)
        w2pairs[:, 0:32, :, 0:C] = lo[0::2]
        w2pairs[:, 32:64, :, C:2 * C] = lo[1::2]
        w2pairs[:, 64:96, :, 0:C] = hi[0::2]
        w2pairs[:, 96:128, :, C:2 * C] = hi[1::2]
        w2pc = w2pairs.reshape(NPAIR * 128, 768).astype(f8)

        b1pc = np.ascontiguousarray(
            np.concatenate([b1[idxA].T, b1[idxB].T], axis=0), dtype=np.float32)
        b2zc = b2[idc] * zc                            # [BS, 64]
        b2zpc = np.ascontiguousarray(
            np.concatenate([b2zc[0::2].T, b2zc[1::2].T], axis=0),
            dtype=np.float32)

        xc4 = np.ascontiguousarray(x[sl]).reshape(BS * C, HW, HW)
        xc = xc4.reshape(BS * C, HW * HW).astype(ml_dtypes.bfloat16)
        # pre-padded relu(x): rows of [0 | relu row | 0] -> 34*32 fp8 cols
        xrp = np.zeros((BS * C, HW, WP), np.float32)
        xrp[:, :, 1:HW + 1] = np.maximum(xc4, 0.0)
        xrc = xrp.reshape(BS * C, WP * HW).astype(ml_dtypes.float8_e4m3)
        out_maps.append(dict(xs=xc, xr=xrc, w1p=w1pc, w2p=w2pc,
                             b1p=b1pc, b2zp=b2zpc))
    return out_maps


def kernel(x, y_index, y_hard, z, W1, b1, W2, b2, _trace=False):
    x = np.asarray(x, dtype=np.float32)
    z = np.asarray(z, dtype=np.float32)
    y_index = np.asarray(y_index)
    W1 = np.asarray(W1, dtype=np.float32)
    b1 = np.asarray(b1, dtype=np.float32)
    W2 = np.asarray(W2, dtype=np.float32)
    b2 = np.asarray(b2, dtype=np.float32)

    nc = _get_nc()
    in_maps = _host_prep(x, y_index, z, W1, b1, W2, b2)
    res = run_bass_kernel_spmd(nc, in_maps, core_ids=list(range(M)), trace=_trace)
    out = np.concatenate(
        [r["out"].astype(np.float32).reshape(BS, C, HW, HW) for r in res.results],
        axis=0)
    if _trace:
        kernel._last_results = res
    return out


# revision 21
# speedup vs baseline: 1.0263x; 1.0213x over previous
"""Trainium2 Bass kernel for nn_DecSwitchedDeconv (switched per-sample double deconv).

Strategy (data-parallel over 8 cores, 32 samples/core, processed in pairs):
  - fp8(e4m3) DoubleRow matmuls: each matmul contracts K=256 (two k-tiles
    summed in one pass) at the same per-column cost as a bf16 K=128 matmul,
    halving tensor-engine work vs bf16. k-tile column strides must be even.
  - conv1 (row-aligned chunks of 11/11/10 image rows): 5 DR matmuls per chunk
    cover 9 taps via tap pairs (delta=2 within a kernel row, delta=34 across);
    conv2: 3 DR matmuls per chunk using the row-shifted hpad duplicate
    (dy0/dy1 in partitions 0:64/64:128, dy2 via +34 column offset with
    zero-masked duplicate taps).
  - Row-aligned chunks let PSUM be read with strided APs that skip the pad
    columns: no pad-column fixup pass, and the conv2 epilogue writes the
    unpadded output tile directly (no opad). Guard columns (+2 front, +4
    back) absorb edge-tap reads of junk output columns.
  - Per-sample branch weights gathered on the HOST into per-pair
    block-diagonal fp8 tables (one contiguous DMA per pair per conv); z is
    folded into W2 and b2*z; biases for all pairs preloaded in one DMA.
  - relu(x) precomputed on the host as pre-padded fp8 rows and DMA'd
    straight into xpad's interior; x also loaded as bf16 for the residual;
    output stored bf16. ScalarE: 3x (bias+relu psum->fp8). VectorE: 3x
    epilogue (psum+b2z) + residual add. hpad row-shift duplicate via
    SBUF->SBUF DMA. Loads + copies on the sync queue (HW DGE), bulk stores
    on the gpsimd queue; PE p-state warmed with dummy matmuls at startup;
    one-iteration software pipeline (conv1(p) overlaps conv2(p-1)).
"""

import numpy as np

import concourse.bacc as bacc
import concourse.bass as bass
import concourse.mybir as mybir
import concourse.tile as tile
from concourse.bass_utils import run_bass_kernel_spmd

B, C, CSM, NB, HW = 256, 64, 32, 8, 32
M = 8                  # cores
BS = B // M            # 32 samples per core
NPAIR = BS // 2        # 16
WP = HW + 2            # 34 padded width
L = WP * WP            # 1156
GD = 2                 # guard columns before the padded image
L2 = L + 6             # guarded tensor width (zeros outside the image)
NBUF = 4               # ping-pong depth for persistent per-pair buffers
CHUNKS = [(1, 11), (12, 11), (23, 10)]   # (first image row, rows) per chunk

f32 = mybir.dt.float32
bf16 = mybir.dt.bfloat16
fp8 = mybir.dt.float8e4

# conv1 DR slots: tap pair (tap0 or None, tap1); rhs tile base is derived
# from tap0's (dy,dx); k-tile delta must be EVEN (hw constraint).
C1SLOTS = [
    (((0, 0), (0, 2)), (0, 0), 2),
    (((1, 0), (1, 2)), (1, 0), 2),
    (((2, 0), (2, 2)), (2, 0), 2),
    (((0, 1), (1, 1)), (0, 1), 34),
    ((None, (2, 1)), (1, 1), 34),   # tile0 zero-weighted
]
# conv2 DR slots: (tile0 lo-position (dy,dx), delta); tiles cover lo tap at
# rows 0:64 and lo+(1,0) at rows 64:128 (the shifted hpad copy)
C2SLOTS = [((0, 0), 2), ((0, 1), 34), ((1, 0), 2)]
# per-tile (lo_tap or None if duplicate-masked, hi_tap)
C2TILES = [((0, 0), (1, 0)), ((0, 2), (1, 2)),
           ((0, 1), (1, 1)), (None, (2, 1)),
           (None, (2, 0)), (None, (2, 2))]


def _build_bass():
    nc = bacc.Bacc(target_bir_lowering=False, debug=False)
    xs = nc.dram_tensor("xs", [BS * C, HW * HW], bf16, kind="ExternalInput")
    xr = nc.dram_tensor("xr", [BS * C, WP * HW], fp8, kind="ExternalInput")
    wcp = nc.dram_tensor("wcp", [NPAIR * 128, 1408], fp8, kind="ExternalInput")
    b1p = nc.dram_tensor("b1p", [64, NPAIR], f32, kind="ExternalInput")
    b2zp = nc.dram_tensor("b2zp", [128, NPAIR], f32, kind="ExternalInput")
    outd = nc.dram_tensor("out", [BS * C, HW * HW], bf16, kind="ExternalOutput")

    add = mybir.AluOpType.add
    Relu = mybir.ActivationFunctionType.Relu
    DR = mybir.MatmulPerfMode.DoubleRow

    with tile.TileContext(nc) as tc:
        b1sb = nc.alloc_sbuf_tensor("b1sb", [64, NPAIR], f32).ap()
        b2zsb = nc.alloc_sbuf_tensor("b2zsb", [128, NPAIR], f32).ap()

        # zero scratch for PE p-state warm-up matmuls
        zdum = nc.alloc_sbuf_tensor("zdum", [128, 384], fp8).ap()
        nc.vector.memset(zdum, 0.0)

        xpads, hpads = [], []
        for i in range(NBUF):
            xpads.append(nc.alloc_sbuf_tensor(f"xpad{i}", [128, L2], fp8).ap())
            hpads.append(nc.alloc_sbuf_tensor(f"hpad{i}", [128, L2], fp8).ap())
            nc.vector.memset(xpads[i], 0.0)
            nc.vector.memset(hpads[i], 0.0)

        with (
            tc.tile_pool(name="io", bufs=4) as iop,
            tc.tile_pool(name="psw", bufs=1, space="PSUM") as pswp,
            tc.tile_pool(name="ps1", bufs=3, space="PSUM") as ps1p,
            tc.tile_pool(name="ps2", bufs=3, space="PSUM") as ps2p,
        ):
            # warm the tensor engine while the first DMAs are in flight
            psw = pswp.tile([64, 376], f32, tag="psw")
            zl = zdum[:, 0:128].rearrange("p (two m) -> p two m", two=2)
            for _ in range(18):
                zr = bass.AP(zdum.tensor, 0, [[384, 128], [2, 2], [1, 376]])
                nc.tensor.matmul(psw[:, :], lhsT=zl, rhs=zr,
                                 start=True, stop=True, perf_mode=DR)

            state = {}

            def emit_dma(p):
                bi = p % NBUF
                xpad, hpad = xpads[bi], hpads[bi]
                xst = iop.tile([128, HW * HW], bf16, tag="xst")
                wtc = iop.tile([128, 1408], fp8, tag="wtc")
                wt1 = wtc[:, 0:640]
                wt2 = wtc[:, 640:1408]
                r0 = p * 128
                # relu(x) arrives pre-padded fp8 from the host, straight into
                # xpad's interior rows (borders stay zero across buffer reuse)
                nc.sync.dma_start(xpad[:, GD + WP:GD + WP + WP * HW],
                                  xr.ap()[r0:r0 + 128, :])
                nc.sync.dma_start(wtc[:, :], wcp.ap()[r0:r0 + 128, :])
                nc.sync.dma_start(xst[:, :], xs.ap()[r0:r0 + 128, :])
                if p == 0:
                    nc.sync.dma_start(b1sb, b1p.ap())
                    nc.sync.dma_start(b2zsb, b2zp.ap())
                xst3 = xst[:, :].rearrange("p (h w) -> p h w", w=HW)
                state[p] = (xpad, hpad, xst, wt1, wt2, xst3)

            def emit_conv1(p):
                xpad, hpad, xst, wt1, wt2, xst3 = state[p]
                hpad3 = hpad[:, GD:GD + L].rearrange("p (h w) -> p h w", w=WP)
                for (y0, rows) in CHUNKS:
                    ncol = WP * rows
                    ps1 = ps1p.tile([64, WP * 11], f32, tag="ps1")
                    for j, (_, t0, delta) in enumerate(C1SLOTS):
                        base = GD + WP * (y0 + t0[0] - 1) + t0[1] - 1
                        rhs = bass.AP(xpad.tensor, base,
                                      [[L2, 128], [delta, 2], [1, ncol]])
                        lhsT = wt1[:, 128 * j:128 * (j + 1)].rearrange(
                            "p (two m) -> p two m", two=2)
                        nc.tensor.matmul(ps1[:, 0:ncol], lhsT=lhsT, rhs=rhs,
                                         start=(j == 0), stop=(j == 4),
                                         perf_mode=DR)
                    ps1v = ps1[:, 0:ncol].rearrange(
                        "p (r w) -> p r w", w=WP)[:, :, 1:HW + 1]
                    nc.scalar.activation(
                        hpad3[0:64, y0:y0 + rows, 1:HW + 1], ps1v,
                        Relu, bias=b1sb[:, p:p + 1])
                # rows 64:128 = h shifted left one image row (dy+1 taps);
                # SBUF->SBUF copy on the sync DMA queue (HW DGE)
                nc.sync.dma_start(hpad[64:128, 0:L2 - WP], hpad[0:64, WP:L2])

            def emit_conv2(p, split_tail=False):
                xpad, hpad, xst, wt1, wt2, xst3 = state.pop(p)
                otmp = iop.tile([128, HW * HW], bf16, tag="otmp")
                ot = iop.tile([128, HW * HW], bf16, tag="ot")
                ot3 = ot[:, :].rearrange("p (h w) -> p h w", w=HW)
                otmp3 = otmp[:, :].rearrange("p (h w) -> p h w", w=HW)
                for (y0, rows) in CHUNKS:
                    ncol = WP * rows
                    ps2 = ps2p.tile([128, WP * 11], f32, tag="ps2")
                    for j, (t0, delta) in enumerate(C2SLOTS):
                        base = GD + WP * (y0 + t0[0] - 1) + t0[1] - 1
                        rhs = bass.AP(hpad.tensor, base,
                                      [[L2, 128], [delta, 2], [1, ncol]])
                        lhsT = wt2[:, 256 * j:256 * (j + 1)].rearrange(
                            "p (two m) -> p two m", two=2)
                        nc.tensor.matmul(ps2[:, 0:ncol], lhsT=lhsT, rhs=rhs,
                                         start=(j == 0), stop=(j == 2),
                                         perf_mode=DR)
                    ps2v = ps2[:, 0:ncol].rearrange(
                        "p (r w) -> p r w", w=WP)[:, :, 1:HW + 1]
                    nc.vector.tensor_scalar(
                        otmp3[:, y0 - 1:y0 - 1 + rows, :], ps2v,
                        b2zsb[:, p:p + 1], None, op0=add)
                    if split_tail:
                        rs = slice(y0 - 1, y0 - 1 + rows)
                        nc.vector.tensor_tensor(
                            ot3[:, rs, :], otmp3[:, rs, :], xst3[:, rs, :],
                            op=add)
                        nc.sync.dma_start(
                            outd.ap()[p * 128:p * 128 + 128,
                                      HW * (y0 - 1):HW * (y0 - 1 + rows)],
                            ot[:, HW * (y0 - 1):HW * (y0 - 1 + rows)])
                if not split_tail:
                    nc.vector.tensor_tensor(ot3, otmp3, xst3, op=add)
                    nc.gpsimd.dma_start(outd.ap()[p * 128:p * 128 + 128, :],
                                        ot[:, :])

            # software pipeline with one-iteration prefetch: DMAs for pair p+1
            # are issued during pair p's conv work, and the tensor queue runs
            # conv1(p) then conv2(p-1), hiding the relu1+copy latency between
            # a pair's two convs
            emit_dma(0)
            for p in range(NPAIR):
                if p + 1 < NPAIR:
                    emit_dma(p + 1)
                emit_conv1(p)
                if p > 0:
                    emit_conv2(p - 1)
            emit_conv2(NPAIR - 1, split_tail=True)

    nc.compile()
    return nc


_NC = None


def _get_nc():
    global _NC
    if _NC is None:
        _NC = _build_bass()
    return _NC


def _host_prep(x, y_index, z, W1, b1, W2, b2):
    import ml_dtypes
    f8 = ml_dtypes.float8_e4m3
    # flipped kernels, tap-indexed [b, cin, dy, dx, cout]
    w1t = np.ascontiguousarray(W1[:, :, :, ::-1, ::-1].transpose(0, 1, 3, 4, 2))
    w2t = np.ascontiguousarray(W2[:, :, :, ::-1, ::-1].transpose(0, 1, 3, 4, 2))

    # conv1 per-branch slot table [NB, cin64, slot5, tile2, cout32]
    base1 = np.zeros((NB, C, 5, 2, CSM), np.float32)
    for j, (taps, _, _) in enumerate(C1SLOTS):
        for i, t in enumerate(taps):
            if t is not None:
                base1[:, :, j, i, :] = w1t[:, :, t[0], t[1], :]

    idx = y_index.reshape(B).astype(np.int64)
    out_maps = []
    for core in range(M):
        sl = slice(core * BS, (core + 1) * BS)
        idc = idx[sl]
        zc = z[sl]
        idxA, idxB = idc[0::2], idc[1::2]

        w1pairs = np.zeros((NPAIR, 128, 5, 2, 2 * CSM), np.float32)
        w1pairs[:, 0:C, :, :, 0:CSM] = base1[idxA]
        w1pairs[:, C:128, :, :, CSM:2 * CSM] = base1[idxB]


        # conv2 per-sample z-folded tables [BS, csm32, tile6, cout64]
        w2g = w2t[idc] * zc[:, None, None, None, :]   # [BS, 32, 3, 3, 64]
        lo = np.zeros((BS, CSM, 6, C), np.float32)
        hi = np.zeros((BS, CSM, 6, C), np.float32)
        for k, (tlo, thi) in enumerate(C2TILES):
            if tlo is not None:
                lo[:, :, k, :] = w2g[:, :, tlo[0], tlo[1], :]
            hi[:, :, k, :] = w2g[:, :, thi[0], thi[1], :]
        w2pairs = np.zeros((NPAIR, 128, 6, 2 * C), np.float32)
        w2pairs[:, 0:32, :, 0:C] = lo[0::2]
        w2pairs[:, 32:64, :, C:2 * C] = lo[1::2]
        w2pairs[:, 64:96, :, 0:C] = hi[0::2]
        w2pairs[:, 96:128, :, C:2 * C] = hi[1::2]
        wcpc = np.concatenate(
            [w1pairs.reshape(NPAIR * 128, 640),
             w2pairs.reshape(NPAIR * 128, 768)], axis=1).astype(f8)

        b1pc = np.ascontiguousarray(
            np.concatenate([b1[idxA].T, b1[idxB].T], axis=0), dtype=np.float32)
        b2zc = b2[idc] * zc                            # [BS, 64]
        b2zpc = np.ascontiguousarray(
            np.concatenate([b2zc[0::2].T, b2zc[1::2].T], axis=0),
            dtype=np.float32)

        xc4 = np.ascontiguousarray(x[sl]).reshape(BS * C, HW, HW)
        xc = xc4.reshape(BS * C, HW * HW).astype(ml_dtypes.bfloat16)
        # pre-padded relu(x): rows of [0 | relu row | 0] -> 34*32 fp8 cols
        xrp = np.zeros((BS * C, HW, WP), np.float32)
        xrp[:, :, 1:HW + 1] = np.maximum(xc4, 0.0)
        xrc = xrp.reshape(BS * C, WP * HW).astype(ml_dtypes.float8_e4m3)
        out_maps.append(dict(xs=xc, xr=xrc, wcp=wcpc,
                             b1p=b1pc, b2zp=b2zpc))
    return out_maps


def kernel(x, y_index, y_hard, z, W1, b1, W2, b2, _trace=False):
    x = np.asarray(x, dtype=np.float32)
    z = np.asarray(z, dtype=np.float32)
    y_index = np.asarray(y_index)
    W1 = np.asarray(W1, dtype=np.float32)
    b1 = np.asarray(b1, dtype=np.float32)
    W2 = np.asarray(W2, dtype=np.float32)
    b2 = np.asarray(b2, dtype=np.float32)

    nc = _get_nc()
    in_maps = _host_prep(x, y_index, z, W1, b1, W2, b2)
    res = run_bass_kernel_spmd(nc, in_maps, core_ids=list(range(M)), trace=_trace)
    out = np.concatenate(
        [r["out"].astype(np.float32).reshape(BS, C, HW, HW) for r in res.results],
        axis=0)
    if _trace:
        kernel._last_results = res
    return out
